# revision 50
# baseline (speedup 1.0000x reference)
"""Masked-attention kernel for 8 TRN2 NeuronCores (batch-parallel sharding).

Per-core shard: 2 batches of [S=2048, D=128] Q/K/V + [S, S] bool mask.
Layout strategy (per core):
  - scores are computed TRANSPOSED (S^T[k, q]) so the PV matmul consumes the
    exp() output directly with V in its natural [k, d] layout.
  - the mask is folded into the scores inside the PE accumulation: an extra
    matmul per (k-tile, q-subtile) with the mask chunk (DMA-cast u8->fp8e4)
    as the stationary operand and a -240*I fp8 identity as the moving
    operand; exp() then flushes masked entries to ~0.
  - softmax denominator: DVE accumulates exp tiles across k-tiles, then per
    q-subtile one [acc-chunk]^T @ ones matmul gives the denominator as a
    PSUM column; reciprocal on DVE; applied as a per-partition scalar after
    the final transpose.
  - Q^T/K^T and O^T->O transposes use single batched DMA-xbar instructions
    (SBUF->SBUF, fp16, per-128-column block transposes).
"""

import numpy as np
import ml_dtypes

B, S, D = 16, 2048, 128
NCORES = 8
BP = B // NCORES  # batches per core
P = 128
QC = 1024  # q-chunk (columns of the transposed score tile)
NQC = S // QC
NKT = S // P  # k tiles
NQS = QC // P  # q subtiles per chunk
MM_N = 512  # matmul moving free dim
SCALE = 1.0 / float(np.sqrt(128.0))
MASK_NEG = -240.0

_CACHE = {}


def build_nc(loop=True):
    import concourse.mybir as mybir
    import concourse.tile as tile
    from concourse import bacc

    fp16 = mybir.dt.float16
    fp32 = mybir.dt.float32

    nc = bacc.Bacc("TRN2", target_bir_lowering=False, debug=False,
                   num_devices=NCORES)

    Qd = nc.dram_tensor("Q", [BP, S, D], fp32, kind="ExternalInput")
    Kd = nc.dram_tensor("K", [BP, S, D], fp32, kind="ExternalInput")
    Vd = nc.dram_tensor("V", [BP, S, D], fp32, kind="ExternalInput")
    Md = nc.dram_tensor("mask", [BP, S, S], mybir.dt.uint8, kind="ExternalInput")
    if loop:
        # run-count knob for differential HW timing (graded path: loop=False)
        Id = nc.dram_tensor("iters", [1, 1], mybir.dt.int32,
                            kind="ExternalInput")
    Od = nc.dram_tensor("out", [BP, S, D], fp32, kind="ExternalOutput")

    negI_np = (MASK_NEG * np.eye(P, dtype=np.float32)).astype(
        ml_dtypes.float8_e4m3)
    negI_dram = nc.inline_tensor(negI_np, name="negI_const")
    ident_dram = nc.inline_tensor(np.eye(P, dtype=np.float16),
                                  name="ident_const")

    with tile.TileContext(nc) as tc:
        with tc.tile_pool(name="consts", bufs=1) as consts, \
             tc.tile_pool(name="stag", bufs=2) as stag, \
             tc.tile_pool(name="qkv", bufs=1) as qkv, \
             tc.tile_pool(name="maskp", bufs=2) as maskp, \
             tc.tile_pool(name="pp", bufs=3) as pp, \
             tc.tile_pool(name="accp", bufs=2) as accp, \
             tc.tile_pool(name="outp", bufs=2) as outp, \
             tc.tile_pool(name="spsum", bufs=2, space="PSUM") as spsum, \
             tc.tile_pool(name="opsum", bufs=1, space="PSUM") as opsum, \
             tc.tile_pool(name="tpsum", bufs=2, space="PSUM") as tpsum:

            negI = consts.tile([P, P], mybir.dt.float8e4)
            nc.sync.dma_start(out=negI[:, :], in_=negI_dram.ap())
            ident = consts.tile([P, P], fp16)
            nc.sync.dma_start(out=ident[:, :], in_=ident_dram.ap())
            ones_col = consts.tile([P, 1], fp16)
            nc.vector.memset(ones_col, 1.0)

            pools = (stag, qkv, maskp, pp, accp, outp, spsum, opsum, tpsum)
            if loop:
                it_sb = consts.tile([1, 1], mybir.dt.int32)
                nc.sync.dma_start(out=it_sb[:, :], in_=Id.ap())
                n_iters = nc.values_load(it_sb[:, :],
                                         skip_runtime_bounds_check=True)
                with tc.For_i(0, n_iters, 1):
                    _kernel_body(nc, mybir, Qd, Kd, Vd, Md, Od, negI,
                                 ident, ones_col, *pools)
            else:
                _kernel_body(nc, mybir, Qd, Kd, Vd, Md, Od, negI,
                             ident, ones_col, *pools)
    nc.compile()
    return nc


def _kernel_body(nc, mybir, Qd, Kd, Vd, Md, Od, negI, ident, ones_col,
                 stag, qkv, maskp, pp, accp, outp, spsum, opsum, tpsum):
    fp16 = mybir.dt.float16
    fp32 = mybir.dt.float32
    fp8 = mybir.dt.float8e4
    Exp = mybir.ActivationFunctionType.Exp

    MC = 512  # mask column-chunk (k) per DMA

    def load_mask(mf, b, qc, cks):
        for ck in cks:
            nc.gpsimd.dma_start(
                out=mf[:, :, ck * MC:(ck + 1) * MC],
                in_=Md.ap()[b, qc * QC:(qc + 1) * QC, ck * MC:(ck + 1) * MC]
                    .rearrange("(s p) k -> p s k", p=P))

    # ---- prefetch the first mask columns before everything else (SWDGE) ----
    mf00 = maskp.tile([P, NQS, S], fp8, name="mf")
    load_mask(mf00, 0, 0, [0, 1])

    # ---- prep: load (HWDGE) + DVE-cast + PE-transpose Q/K, load V ----
    HT = NKT // 2  # tiles per half-load

    def load_cast_transpose_half(src_ap, dst, b, h, ring):
        # load a [S/2, D] f32 half, cast to fp16, PE-transpose each 128x128
        # tile into a 1-bank PSUM staging slot, copy back to dst [d, s].
        f = stag.tile([P, HT, D], fp32, name="ldf")
        ring(out=f[:, :, :],
             in_=src_ap[b, h * HT * P:(h + 1) * HT * P, :]
                 .rearrange("(t p) d -> p t d", p=P))
        g = stag.tile([P, HT, D], fp16, name="ldh")
        nc.vector.tensor_copy(out=g[:, :, :], in_=f[:, :, :])
        tps = tpsum.tile([P, HT * P], fp16, name="tps")
        for t in range(HT):
            nc.tensor.transpose(tps[:, t * P:(t + 1) * P],
                                g[:, t, :], ident[:, :])
        nc.vector.tensor_copy(
            out=dst[:, h * HT * P:(h + 1) * HT * P], in_=tps[:, :])

    qts, ktts, vsbs = [], [], []
    for b in range(BP):
        ktt = qkv.tile([P, S], fp16, name=f"ktt{b}")
        qt = qkv.tile([P, S], fp16, name=f"qt{b}")
        for h in range(2):
            load_cast_transpose_half(Kd.ap(), ktt, b, h, nc.sync.dma_start)
            load_cast_transpose_half(Qd.ap(), qt, b, h, nc.scalar.dma_start)
        if b == 0:
            load_mask(mf00, 0, 0, [2, 3])
        vf = stag.tile([P, S], fp32, name="vf")
        nc.sync.dma_start(
            out=vf[:, :].rearrange("p (t d) -> p t d", t=NKT),
            in_=Vd.ap()[b].rearrange("(t p) d -> p t d", p=P))
        vsb = qkv.tile([P, NKT, D], fp16, name=f"vsb{b}")
        nc.vector.tensor_copy(
            out=vsb[:, :, :],
            in_=vf[:, :].rearrange("p (t d) -> p t d", t=NKT))
        qts.append(qt)
        ktts.append(ktt)
        vsbs.append(vsb)

    # ---- main flash loop over (batch, q-chunk, k-tile) ----
    for b in range(BP):
        qt, ktt, vsb = qts[b], ktts[b], vsbs[b]
        for qc in range(NQC):
            if b == 0 and qc == 0:
                mf = mf00
            else:
                mf = maskp.tile([P, NQS, S], fp8, name="mf")
                load_mask(mf, b, qc, range(S // MC))
            acc = accp.tile([P, QC], fp16, name="acc")
            ops = opsum.tile([P, QC], fp32, name="opsum")
            for kt in range(NKT):
                sc = spsum.tile([P, QC], fp32, name="scores")
                for n in range(0, QC, MM_N):
                    nc.tensor.matmul(
                        sc[:, n:n + MM_N],
                        lhsT=ktt[:, kt * P:(kt + 1) * P],
                        rhs=qt[:, qc * QC + n:qc * QC + n + MM_N],
                        start=True, stop=False, skip_group_check=True)
                for sq in range(NQS):
                    nc.tensor.matmul(
                        sc[:, sq * P:(sq + 1) * P],
                        lhsT=mf[:, sq, kt * P:(kt + 1) * P],
                        rhs=negI[:, :],
                        start=False,
                        stop=(sq % (MM_N // P) == MM_N // P - 1),
                        skip_group_check=True)
                pt = pp.tile([P, QC], fp16, name="pt")
                nc.scalar.activation(out=pt[:, :], in_=sc[:, :],
                                     func=Exp, scale=SCALE)
                if kt == 0:
                    nc.vector.tensor_copy(out=acc[:, :], in_=pt[:, :])
                else:
                    nc.vector.tensor_add(out=acc[:, :], in0=acc[:, :],
                                         in1=pt[:, :])
                for n in range(0, QC, MM_N):
                    nc.tensor.matmul(
                        ops[:, n:n + MM_N],
                        lhsT=vsb[:, kt, :],
                        rhs=pt[:, n:n + MM_N],
                        start=(kt == 0), stop=(kt == NKT - 1),
                        skip_group_check=True)

            # denominator as a PSUM column per q-subtile:
            # den[q_local, sq] = sum_k acc[k, sq*128 + q_local]
            den = opsum.tile([P, NQS], fp32, name="opsum")
            for sq in range(NQS):
                nc.tensor.matmul(den[:, sq:sq + 1],
                                 lhsT=acc[:, sq * P:(sq + 1) * P],
                                 rhs=ones_col[:, :],
                                 start=True, stop=True,
                                 skip_group_check=True)
            rcol = outp.tile([P, NQS], fp32, name="rcol")
            nc.vector.reciprocal(out=rcol[:, :], in_=den[:, :])

            ot = outp.tile([P, QC], fp16, name="ot")
            nc.vector.tensor_copy(out=ot[:, :], in_=ops[:, :])
            osb = tpsum.tile([P, QC], fp16, name="tps")
            for t in range(NQS):
                nc.tensor.transpose(osb[:, t * P:(t + 1) * P],
                                    ot[:, t * P:(t + 1) * P], ident[:, :])
            osf = outp.tile([P, NQS, D], fp32, name="osf")
            for t in range(NQS):
                nc.vector.tensor_scalar_mul(
                    out=osf[:, t, :],
                    in0=osb[:, t * P:(t + 1) * P],
                    scalar1=rcol[:, t:t + 1])
            nc.scalar.dma_start(
                out=Od.ap()[b, qc * QC:(qc + 1) * QC, :]
                    .rearrange("(t p) d -> p t d", p=P),
                in_=osf[:, :, :])


def _get_nc(loop=False):
    key = f"nc_loop{loop}"
    if key not in _CACHE:
        _CACHE[key] = build_nc(loop=loop)
    return _CACHE[key]


def kernel(Q, K, V, mask, dk=128):
    from concourse.bass_utils import run_bass_kernel_spmd

    assert int(dk) == 128
    Q = np.ascontiguousarray(np.asarray(Q, dtype=np.float32))
    K = np.ascontiguousarray(np.asarray(K, dtype=np.float32))
    V = np.ascontiguousarray(np.asarray(V, dtype=np.float32))
    mask_u8 = np.ascontiguousarray(np.asarray(mask)).astype(np.uint8)

    nc = _get_nc(loop=False)
    in_maps = []
    for c in range(NCORES):
        sl = slice(c * BP, (c + 1) * BP)
        in_maps.append({
            "Q": np.ascontiguousarray(Q[sl]),
            "K": np.ascontiguousarray(K[sl]),
            "V": np.ascontiguousarray(V[sl]),
            "mask": np.ascontiguousarray(mask_u8[sl]),
        })
    res = run_bass_kernel_spmd(nc, in_maps, core_ids=list(range(NCORES)))
    return np.concatenate([r["out"] for r in res.results], axis=0)
